# revision 4
# baseline (speedup 1.0000x reference)
"""BrainModel kernel for 8 TRN2 NeuronCores (raw bass, no Tile).

Reference computation:
    gathered = x[:, idx]                              # [B, O, C]
    pre = einsum('boc,oc->bo', gathered, w_sparse) + b_sparse
    new_x = sigmoid(pre)                              # [B, O]
    q = new_x[:, -N_MOTORS:] @ w_motor.T + b_motor    # [B, A]

Only the last N_MOTORS=256 rows of idx/w_sparse/b_sparse reach q, so the
other 98720 output neurons are dead code. We shard those 256 motor
neurons across the 8 cores (32 each -> 1024 gathered x-columns per core).

v2: the per-core gather is 4 dma_gather instructions (SWDGE custom op,
994ns fixed + 0.34ns/descriptor) instead of 8 indirect DMAs (994ns fixed
EACH, one index per partition). dma_gather indices are int16, so neuron
ids (< 100000) don't fit; we split indices by n%4 and gather residue r
from base offset r*256B with row stride 1024B, so the key n//4 <= 24999
fits int16. The x table is bf16 [100000, 128] (batch padded 64->128 to
satisfy the 256B-elem-size constraint), weights are bf16 -> single-pass
matmuls (no fp32 LOW/HIGH dual pumping).

Per-core device program:
  Scalar: HWDGE load of the int16 gather keys (aux1) ASAP; sigmoid LUT
     preload; later sigmoid(pre)+bias -> bf16, identity(q)+bias, out DMA.
  Sync:   HWDGE load of packed weights (aux2).
  Vector: zero G (gather dst) so -1-padded key slots stay 0.
  GpSimd: load_library(mlp) [dma_gather ucode], warmup dma_gather (Q7
     code/stall warm), then 4 real gathers of ~256 rows x 256B each.
  Tensor: 12 accumulating bf16 matmuls -> pre [32, B]; motor matmul
     -> q partial [A, B].
Host sums the 8 partials and transposes to [B, A].
"""

from contextlib import ExitStack

import ml_dtypes
import numpy as np

import concourse.bass as bass
from concourse import mybir
from concourse.library_config import mlp
from concourse.library_overlay import lower_extended_insts

N_NEURONS = 100000
N_MOTORS = 256
N_CONN = 32
N_ACT = 16
BATCH = 64
N_CORES = 8
M_PER_CORE = N_MOTORS // N_CORES  # 32 motor neurons per core
R = M_PER_CORE * N_CONN  # 1024 gathered x-rows per core
P = 128  # SBUF partitions

NRES = 4  # residue classes (n % 4)
KROWS = N_NEURONS // NRES  # 25000 int16-addressable table rows per residue
TBL_W = 128  # bf16 row: 64 batch + 64 pad = 256B (elem-size constraint)
PAD_N = 384  # per-gather index slots (3 x 128); counts are ~256 +- 40
BLKS_G = PAD_N // P  # 3 G blocks per gather
T = NRES * BLKS_G  # 12 matmul chunks
IDX_C16 = PAD_N // 16  # 24 int16 idx cols per gather
AUX1C = NRES * IDX_C16 // 2  # 48 f32 cols of int16 keys

C_WMT = T * M_PER_CORE // 2  # 192: f32 cols of bf16 Wk
C_BS = C_WMT + N_ACT // 2  # 200: b_sparse col
C_BM = C_BS + 1  # 201: b_motor/8 col
AUX2C = C_BM + 1  # 202

BF16 = ml_dtypes.bfloat16

_CACHE: dict = {}


def _build_nc(key) -> bass.Bass:
    """key = (perm, (N_0..N_3)) - residue order and uniform per-gather
    real-index counts (num_idxs_reg is baked into the instruction stream)."""
    perm, ng = key
    f32 = mybir.dt.float32
    bf16 = mybir.dt.bfloat16
    i16 = mybir.dt.int16
    nc = bass.Bass(enable_partition_id=False)

    tbl = nc.declare_dram_parameter("tbl", [KROWS, NRES * TBL_W], bf16, isOutput=False)
    aux1 = nc.declare_dram_parameter("aux1", [P, AUX1C], f32, isOutput=False)
    aux2 = nc.declare_dram_parameter("aux2", [P, AUX2C], f32, isOutput=False)
    out = nc.declare_dram_parameter("out", [N_ACT, BATCH], f32, isOutput=True)

    with ExitStack() as ctx:
        aux1_sb = ctx.enter_context(nc.sbuf_tensor("aux1_sb", [P, AUX1C], f32))
        aux2_sb = ctx.enter_context(nc.sbuf_tensor("aux2_sb", [P, AUX2C], f32))
        G = ctx.enter_context(nc.sbuf_tensor("G", [P, T, TBL_W], bf16))
        wscr = ctx.enter_context(nc.sbuf_tensor("wscr", [P, 1, TBL_W], bf16))
        widx = ctx.enter_context(nc.sbuf_tensor("widx", [P, 1], i16))
        s_sb = ctx.enter_context(nc.sbuf_tensor("s_sb", [M_PER_CORE, BATCH], bf16))
        q_sb = ctx.enter_context(nc.sbuf_tensor("q_sb", [N_ACT, BATCH], f32))
        pre_ps = ctx.enter_context(nc.psum_tensor("pre_ps", [M_PER_CORE, BATCH], f32))
        q_ps = ctx.enter_context(nc.psum_tensor("q_ps", [N_ACT, BATCH], f32))
        isem = ctx.enter_context(nc.semaphore("isem"))
        wsem = ctx.enter_context(nc.semaphore("wsem"))
        zsem = ctx.enter_context(nc.semaphore("zsem"))
        wgsem = ctx.enter_context(nc.semaphore("wgsem"))
        odma_sem = ctx.enter_context(nc.semaphore("odma_sem"))
        gsems = [ctx.enter_context(nc.semaphore(f"gsem{g}")) for g in range(NRES)]
        pe_sem = ctx.enter_context(nc.semaphore("pe_sem"))
        act_sem = ctx.enter_context(nc.semaphore("act_sem"))
        warm_sb = ctx.enter_context(nc.sbuf_tensor("warm_sb", [1, 1], f32))
        pad_sb = ctx.enter_context(nc.sbuf_tensor("pad_sb", [1, 1], f32))
        block = ctx.enter_context(nc.Block())

        @block.scalar
        def _(scalar):
            # keys first, HWDGE from Scalar (frees Sync for the weights and
            # starts ~0.25us earlier than Sync's post-barrier slot).
            scalar.dma_start(out=aux1_sb[:], in_=aux1[:]).then_inc(isem, 16)
            # Dummy activation preloads the sigmoid LUT off the critical path
            # (~1.3us table load). Reads its own scratch; value is irrelevant.
            scalar.activation(
                warm_sb[:], warm_sb[:], mybir.ActivationFunctionType.Sigmoid
            )
            scalar.wait_ge(pe_sem, 1)
            # s = sigmoid(pre + b_sparse), bf16 out for the 1-pass motor matmul
            scalar.activation(
                s_sb[:],
                pre_ps[:],
                mybir.ActivationFunctionType.Sigmoid,
                bias=aux2_sb[:M_PER_CORE, C_BS : C_BS + 1],
            ).then_inc(act_sem, 1)
            scalar.wait_ge(pe_sem, 2)
            # q_sb = q_ps + b_motor/8 (PSUM -> SBUF)
            scalar.activation(
                q_sb[:],
                q_ps[:],
                mybir.ActivationFunctionType.Identity,
                bias=aux2_sb[:N_ACT, C_BM : C_BM + 1],
            )
            scalar.dma_start(out=out[:], in_=q_sb[:]).then_inc(odma_sem, 16)

        @block.sync
        def _(sync):
            sync.dma_start(out=aux2_sb[:], in_=aux2[:]).then_inc(wsem, 16)
            sync.wait_ge(odma_sem, 16)

        @block.vector
        def _(vector):
            # G slots that no gather writes (padded tails) must be 0, not
            # stale SBUF (0 x NaN would poison PSUM).
            vector.memset(G[:], 0).then_inc(zsem, 1)

        @block.gpsimd
        def _(gpsimd):
            # dma_gather lives in the mlp Q7 library; load it while the key
            # DMA is in flight.
            gpsimd.load_library(mlp)
            gpsimd.memset(widx[:], 0)
            # Warmup gather: exercises the freshly loaded dma_gather path so
            # the first real gather doesn't eat the cold-start stall.
            gpsimd.dma_gather(
                wscr[:],
                tbl[:, 0:TBL_W],
                widx[:],
                16,
                16,
                TBL_W,
                elem_step=NRES * TBL_W,
            ).then_inc(wgsem, 16)
            gpsimd.wait_ge(zsem, 1)
            gpsimd.wait_ge(isem, 16)
            # cheap op after the waits absorbs the Pool post-wait dispatch
            # stall so it doesn't land on the first gather.
            gpsimd.memset(pad_sb[:], 0)
            for g in range(NRES):
                r = perm[g]
                gpsimd.dma_gather(
                    G[:, g * BLKS_G : (g + 1) * BLKS_G, :],
                    tbl[:, r * TBL_W : (r + 1) * TBL_W],
                    aux1_sb[:, g * IDX_C16 // 2 : (g + 1) * IDX_C16 // 2].bitcast(i16),
                    PAD_N,
                    ng[g],
                    TBL_W,
                    elem_step=NRES * TBL_W,
                ).then_inc(gsems[g], 16)

        @block.tensor
        def _(tensor):
            tensor.wait_ge(wsem, 16)
            wk = aux2_sb[:, :C_WMT].bitcast(mybir.dt.bfloat16)  # [128, T*32]
            for g in range(NRES):
                tensor.wait_ge(gsems[g], 16)
                for k in range(BLKS_G):
                    t = g * BLKS_G + k
                    mm = tensor.matmul(
                        pre_ps[:],
                        wk[:, t * M_PER_CORE : (t + 1) * M_PER_CORE],
                        G[:, t, :BATCH],
                        start=(t == 0),
                        stop=(t == T - 1),
                    )
            mm.then_inc(pe_sem, 1)
            tensor.wait_ge(act_sem, 1)
            # q_part[a, b] = sum_m wmT[m, a] * s[m, b]
            tensor.matmul(
                q_ps[:],
                aux2_sb[:M_PER_CORE, C_WMT : C_WMT + N_ACT // 2].bitcast(
                    mybir.dt.bfloat16
                ),
                s_sb[:],
                start=True,
                stop=True,
            ).then_inc(pe_sem, 1)

    # Raw Bass skips Bacc's codegen_inst_isa_subclasses pass; without it the
    # InstPseudoReloadLibraryIndex reaches walrus with empty .instr bytes
    # ("ISA wrong length").
    lower_extended_insts(nc)
    return nc


def _get_nc(key) -> bass.Bass:
    if _CACHE.get("nc_key") != key:
        _CACHE["nc"] = _build_nc(key)
        _CACHE["nc_key"] = key
    return _CACHE["nc"]


def make_in_maps(x, idx, w_sparse, b_sparse, w_motor, b_motor):
    """Shard FULL inputs into the 8 per-core input dicts + the nc cache key."""
    x = np.asarray(x, dtype=np.float32)
    idx_m = np.asarray(idx)[-N_MOTORS:].astype(np.int64)  # [256, 32]
    w_m = np.asarray(w_sparse, dtype=np.float32)[-N_MOTORS:]  # [256, 32]
    b_m = np.asarray(b_sparse, dtype=np.float32)[-N_MOTORS:]  # [256]
    wm = np.asarray(w_motor, dtype=np.float32)  # [16, 256]
    bm = np.asarray(b_motor, dtype=np.float32)  # [16]

    # bf16 table, batch padded 64 -> 128 so a row is 256B; row n = x[:, n].
    tbl = np.zeros((N_NEURONS, TBL_W), dtype=BF16)
    tbl[:, :BATCH] = x.T.astype(BF16)
    tbl = np.ascontiguousarray(tbl).reshape(KROWS, NRES * TBL_W)

    # Per (core, residue) entry lists: (key=n//4, motor, weight).
    flat_n = idx_m.reshape(N_CORES, R)  # core k -> 1024 neuron ids
    flat_w = w_m.reshape(N_CORES, R)
    res = flat_n % NRES
    keys = flat_n // NRES
    counts = np.zeros((N_CORES, NRES), np.int64)
    for k in range(N_CORES):
        for r in range(NRES):
            counts[k, r] = int((res[k] == r).sum())
    ncounts = counts.max(axis=0)  # uniform num_idxs_reg per gather
    assert (ncounts <= PAD_N).all(), ncounts
    # biggest first -> smallest gather last on the critical path
    perm = tuple(int(r) for r in np.argsort(-ncounts, kind="stable"))
    ng = tuple(int(ncounts[r]) for r in perm)

    in_maps = []
    for k in range(N_CORES):
        aux1_i16 = np.full((16, NRES * IDX_C16), -1, np.int16)
        wk = np.zeros((P, T * M_PER_CORE), BF16)
        for g, r in enumerate(perm):
            sel = np.nonzero(res[k] == r)[0]  # flat positions m*32+c
            n_real = len(sel)
            kk = keys[k, sel].astype(np.int16)
            # pad to the uniform count with key 0 (valid row, zero weight)
            kk = np.concatenate([kk, np.zeros(ng[g] - n_real, np.int16)])
            i = np.arange(ng[g])
            aux1_i16[i % 16, g * IDX_C16 + i // 16] = kk
            # gathered row i lands at G[i%128, g*3 + i//128, :]; its weight
            # multiplies motor column m in matmul chunk t = g*3 + i//128
            m = sel // N_CONN
            wk[i[:n_real] % P, (g * BLKS_G + i[:n_real] // P) * M_PER_CORE + m] = (
                flat_w[k, sel].astype(BF16)
            )
        aux1 = np.ascontiguousarray(np.tile(aux1_i16, (8, 1))).view(np.float32)

        rows = slice(k * M_PER_CORE, (k + 1) * M_PER_CORE)
        aux2 = np.zeros((P, AUX2C), np.float32)
        aux2[:, :C_WMT] = wk.view(np.float32)
        aux2[:M_PER_CORE, C_WMT:C_BS] = (
            np.ascontiguousarray(wm[:, rows].T.astype(BF16))
            .view(np.float32)
            .reshape(M_PER_CORE, N_ACT // 2)
        )
        aux2[:M_PER_CORE, C_BS] = b_m[rows]
        aux2[:N_ACT, C_BM] = bm / N_CORES

        in_maps.append({"tbl": tbl, "aux1": aux1, "aux2": aux2})
    return in_maps, (perm, ng)


def combine_outputs(partials):
    """Reduce the 8 per-core [A, B] partials to the full [B, A] output."""
    q = np.sum(np.stack(partials, axis=0), axis=0, dtype=np.float64)
    return np.ascontiguousarray(q.T).astype(np.float32)


def _ensure_trace_hook_importable():
    """bass_utils' axon trace path imports antenv.axon_hooks; some containers
    ship an antenv without it. Provide a null hook so trace degrades to a
    plain run instead of crashing."""
    import os

    if not os.environ.get("BASS_TRACE"):
        return
    try:
        import antenv.axon_hooks  # noqa: F401
    except ImportError:
        import sys
        import types

        import antenv

        m = types.ModuleType("antenv.axon_hooks")
        state = {"hook": None}
        m.set_axon_ntff_profile_hook = lambda h: state.__setitem__("hook", h)
        m.get_axon_ntff_profile_hook = lambda: state["hook"]
        sys.modules["antenv.axon_hooks"] = m
        antenv.axon_hooks = m


def kernel(x, idx, w_sparse, b_sparse, w_motor, b_motor):
    from concourse.bass_utils import run_bass_kernel_spmd

    _ensure_trace_hook_importable()
    in_maps, key = make_in_maps(x, idx, w_sparse, b_sparse, w_motor, b_motor)
    nc = _get_nc(key)
    res = run_bass_kernel_spmd(nc, in_maps, core_ids=list(range(N_CORES)))
    _CACHE["last_results"] = res
    return combine_outputs([res.results[k]["out"] for k in range(N_CORES)])


# revision 6
# speedup vs baseline: 1.5155x; 1.5155x over previous
"""BrainModel kernel for 8 TRN2 NeuronCores (raw bass, no Tile).

Reference computation:
    gathered = x[:, idx]                              # [B, O, C]
    pre = einsum('boc,oc->bo', gathered, w_sparse) + b_sparse
    new_x = sigmoid(pre)                              # [B, O]
    q = new_x[:, -N_MOTORS:] @ w_motor.T + b_motor    # [B, A]

Only the last N_MOTORS=256 rows of idx/w_sparse/b_sparse reach q, so the
other 98720 output neurons are dead code. We shard those 256 motor
neurons across the 8 cores (32 each -> 1024 gathered x-columns per core).

The gather is 8 SWDGE indirect DMAs of 128 descriptors each (measured HW
facts: the indirect path consumes exactly ONE index per partition per
instruction - multi-column offset APs are ignored/garbage, dma_gather's
Q7 ucode is 30x slower per descriptor plus a 9us library load, and HWDGE
engines crash on indirect DMAs - so 8 x ~1.1us serialized emission on
the Pool sequencer is the floor).

v4 over the original baseline:
  - x table in bf16 [100000, 64]: gather descriptors move 128B instead
    of 256B, and all matmuls run single-pass bf16 (the fp32 path pumps
    every matmul twice as LOW/HIGH). rel err ~3e-3, tolerance is 2e-2.
  - two warmup gathers (row 0 x 128 into scratch) keep the Q7 SWDGE
    busy from preamble exit until the index tile lands, eating the
    SWDGE cold-start + post-wait dispatch stall (~1-2us combined).
  - sigmoid outputs bf16 so the motor matmul is also single-pass.
Host sums the 8 partials and transposes to [B, A].
"""

from contextlib import ExitStack

import ml_dtypes
import numpy as np

import concourse.bass as bass
from concourse import mybir

N_NEURONS = 100000
N_MOTORS = 256
N_CONN = 32
N_ACT = 16
BATCH = 64
N_CORES = 8
M_PER_CORE = N_MOTORS // N_CORES  # 32 motor neurons per core
R = M_PER_CORE * N_CONN  # 1024 gathered x-rows per core
P = 128  # SBUF partitions
J = R // P  # 8 gather/matmul chunks

C_WK = J * M_PER_CORE // 2  # 128: f32 cols of bf16 Wk
C_WMT = C_WK + N_ACT // 2  # 136: wmT end (bf16 [32, 16])
C_BS = C_WMT  # 136: b_sparse col
C_BM = C_BS + 1  # 137: b_motor/8 col
C_IDX = C_BM + 1  # 138: idx cols (8 x int32)
AUXC = C_IDX + J  # 146

BF16 = ml_dtypes.bfloat16

_CACHE: dict = {}


def _build_nc() -> bass.Bass:
    f32 = mybir.dt.float32
    bf16 = mybir.dt.bfloat16
    i32 = mybir.dt.int32
    nc = bass.Bass(enable_partition_id=False)

    tbl = nc.declare_dram_parameter("tbl", [N_NEURONS, BATCH], bf16, isOutput=False)
    aux = nc.declare_dram_parameter("aux", [P, AUXC], f32, isOutput=False)
    out = nc.declare_dram_parameter("out", [N_ACT, BATCH], f32, isOutput=True)

    with ExitStack() as ctx:
        aux_sb = ctx.enter_context(nc.sbuf_tensor("aux_sb", [P, AUXC], f32))
        G = ctx.enter_context(nc.sbuf_tensor("G", [P, J * BATCH], bf16))
        wscr = ctx.enter_context(nc.sbuf_tensor("wscr", [P, BATCH], bf16))
        widx = ctx.enter_context(nc.sbuf_tensor("widx", [P, 1], i32))
        s_sb = ctx.enter_context(nc.sbuf_tensor("s_sb", [M_PER_CORE, BATCH], bf16))
        q_sb = ctx.enter_context(nc.sbuf_tensor("q_sb", [N_ACT, BATCH], f32))
        pre_ps = ctx.enter_context(nc.psum_tensor("pre_ps", [M_PER_CORE, BATCH], f32))
        q_ps = ctx.enter_context(nc.psum_tensor("q_ps", [N_ACT, BATCH], f32))
        isem = ctx.enter_context(nc.semaphore("isem"))
        wsem = ctx.enter_context(nc.semaphore("wsem"))
        wgsem = ctx.enter_context(nc.semaphore("wgsem"))
        odma_sem = ctx.enter_context(nc.semaphore("odma_sem"))
        # One completion sem per gather chunk: a single shared sem would be
        # racy -- each DMA's 16 increments come from 16 independent SDMA
        # engines, so a running count can reach 16*(j+1) before chunk j has
        # fully landed.
        gdma_sems = [
            ctx.enter_context(nc.semaphore(f"gdma_sem{j}")) for j in range(J)
        ]
        pe_sem = ctx.enter_context(nc.semaphore("pe_sem"))
        act_sem = ctx.enter_context(nc.semaphore("act_sem"))
        warm_sb = ctx.enter_context(nc.sbuf_tensor("warm_sb", [1, 1], f32))
        pad_sb = ctx.enter_context(nc.sbuf_tensor("pad_sb", [1, 1], f32))
        block = ctx.enter_context(nc.Block())

        @block.sync
        def _(sync):
            # idx columns first (small) so the gathers start ASAP; weights on
            # their own sem (completion order of two DMAs is not guaranteed).
            sync.dma_start(
                out=aux_sb[:, C_IDX:AUXC], in_=aux[:, C_IDX:AUXC]
            ).then_inc(isem, 16)
            sync.dma_start(out=aux_sb[:, :C_IDX], in_=aux[:, :C_IDX]).then_inc(
                wsem, 16
            )
            sync.wait_ge(odma_sem, 16)

        @block.gpsimd
        def _(gpsimd):
            # Two warmup gathers (row 0 into scratch) keep the Q7 SWDGE busy
            # from preamble exit until the idx tile lands: the first one eats
            # the SWDGE cold-start, the second keeps Q7 hot across the isem
            # wait so the post-wait dispatch stall shrinks.
            gpsimd.memset(widx[:], 0)
            for _w in range(2):
                gpsimd.indirect_dma_start(
                    out=wscr[:],
                    out_offset=None,
                    in_=tbl[:],
                    in_offset=bass.IndirectOffsetOnAxis(ap=widx[:], axis=0),
                ).then_inc(wgsem, 16)
            gpsimd.wait_ge(isem, 16)
            # Cheap op right after the wait absorbs part of the Pool post-wait
            # dispatch stall.
            gpsimd.memset(pad_sb[:], 0)
            # One index per partition per instruction: partition p of the dest
            # gets dest-free-size contiguous bytes starting at tbl row idx[p].
            for j in range(J):
                gpsimd.indirect_dma_start(
                    out=G[:, j * BATCH : (j + 1) * BATCH],
                    out_offset=None,
                    in_=tbl[:],
                    in_offset=bass.IndirectOffsetOnAxis(
                        ap=aux_sb[:, C_IDX + j : C_IDX + j + 1].bitcast(i32),
                        axis=0,
                    ),
                ).then_inc(gdma_sems[j], 16)

        @block.tensor
        def _(tensor):
            tensor.wait_ge(wsem, 16)
            wk = aux_sb[:, :C_WK].bitcast(mybir.dt.bfloat16)  # [128, J*32]
            # pre[m, b] = sum_{p,j} Wk[p, j*32+m] * x[b, idx_flat[p*J+j]]
            for j in range(J):
                tensor.wait_ge(gdma_sems[j], 16)
                mm = tensor.matmul(
                    pre_ps[:],
                    wk[:, j * M_PER_CORE : (j + 1) * M_PER_CORE],
                    G[:, j * BATCH : (j + 1) * BATCH],
                    start=(j == 0),
                    stop=(j == J - 1),
                )
            mm.then_inc(pe_sem, 1)
            tensor.wait_ge(act_sem, 1)
            # q_part[a, b] = sum_m wmT[m, a] * s[m, b]
            tensor.matmul(
                q_ps[:],
                aux_sb[:M_PER_CORE, C_WK:C_WMT].bitcast(mybir.dt.bfloat16),
                s_sb[:],
                start=True,
                stop=True,
            ).then_inc(pe_sem, 1)

        @block.scalar
        def _(scalar):
            # Dummy activation preloads the sigmoid LUT off the critical path
            # (the table load is ~1.3us and otherwise serializes after the
            # last matmul). Reads its own scratch; the value is irrelevant.
            scalar.activation(
                warm_sb[:], warm_sb[:], mybir.ActivationFunctionType.Sigmoid
            )
            scalar.wait_ge(pe_sem, 1)
            # s = sigmoid(pre + b_sparse), bf16 out for the 1-pass motor matmul
            scalar.activation(
                s_sb[:],
                pre_ps[:],
                mybir.ActivationFunctionType.Sigmoid,
                bias=aux_sb[:M_PER_CORE, C_BS : C_BS + 1],
            ).then_inc(act_sem, 1)
            scalar.wait_ge(pe_sem, 2)
            # q_sb = q_ps + b_motor/8 (PSUM -> SBUF)
            scalar.activation(
                q_sb[:],
                q_ps[:],
                mybir.ActivationFunctionType.Identity,
                bias=aux_sb[:N_ACT, C_BM : C_BM + 1],
            )
            # ScalarE is HWDGE-capable: issue the output DMA right here,
            # skipping a cross-engine semaphore hop to Sync.
            scalar.dma_start(out=out[:], in_=q_sb[:]).then_inc(odma_sem, 16)

    return nc


def _get_nc() -> bass.Bass:
    if "nc" not in _CACHE:
        _CACHE["nc"] = _build_nc()
    return _CACHE["nc"]


def make_in_maps(x, idx, w_sparse, b_sparse, w_motor, b_motor):
    """Shard FULL inputs into the 8 per-core input dicts."""
    x = np.asarray(x, dtype=np.float32)
    idx_m = np.asarray(idx)[-N_MOTORS:].astype(np.int32)  # [256, 32]
    w_m = np.asarray(w_sparse, dtype=np.float32)[-N_MOTORS:]  # [256, 32]
    b_m = np.asarray(b_sparse, dtype=np.float32)[-N_MOTORS:]  # [256]
    wm = np.asarray(w_motor, dtype=np.float32)  # [16, 256]
    bm = np.asarray(b_motor, dtype=np.float32)  # [16]

    tbl = np.ascontiguousarray(x.T.astype(BF16))  # [N_NEURONS, B], row n = x[:, n]

    # flat position r = p*J + j: gathered row lands at G[p, j*B:(j+1)*B]; its
    # weight multiplies motor column m = r//32 of matmul chunk j.
    r = np.arange(R)
    o_l, c = r // N_CONN, r % N_CONN
    p_r, j_r = r // J, r % J

    in_maps = []
    for k in range(N_CORES):
        rows = slice(k * M_PER_CORE, (k + 1) * M_PER_CORE)
        w_core = w_m[rows]  # [32, 32]

        aux = np.zeros((P, AUXC), np.float32)
        wk = np.zeros((P, J * M_PER_CORE), BF16)
        wk[p_r, j_r * M_PER_CORE + o_l] = w_core[o_l, c].astype(BF16)
        aux[:, :C_WK] = wk.view(np.float32)
        aux[:M_PER_CORE, C_WK:C_WMT] = (
            np.ascontiguousarray(wm[:, rows].T.astype(BF16))
            .view(np.float32)
            .reshape(M_PER_CORE, N_ACT // 2)
        )
        aux[:M_PER_CORE, C_BS] = b_m[rows]
        aux[:N_ACT, C_BM] = bm / N_CORES
        idx_tile = np.ascontiguousarray(idx_m[rows].reshape(P, J))  # int32
        aux[:, C_IDX:AUXC] = idx_tile.view(np.float32)

        in_maps.append({"tbl": tbl, "aux": aux})
    return in_maps


def combine_outputs(partials):
    """Reduce the 8 per-core [A, B] partials to the full [B, A] output."""
    q = np.sum(np.stack(partials, axis=0), axis=0, dtype=np.float64)
    return np.ascontiguousarray(q.T).astype(np.float32)


def _ensure_trace_hook_importable():
    """bass_utils' axon trace path imports antenv.axon_hooks; some containers
    ship an antenv without it. Provide a null hook so trace degrades to a
    plain run instead of crashing."""
    import os

    if not os.environ.get("BASS_TRACE"):
        return
    try:
        import antenv.axon_hooks  # noqa: F401
    except ImportError:
        import sys
        import types

        import antenv

        m = types.ModuleType("antenv.axon_hooks")
        state = {"hook": None}
        m.set_axon_ntff_profile_hook = lambda h: state.__setitem__("hook", h)
        m.get_axon_ntff_profile_hook = lambda: state["hook"]
        sys.modules["antenv.axon_hooks"] = m
        antenv.axon_hooks = m


def kernel(x, idx, w_sparse, b_sparse, w_motor, b_motor):
    from concourse.bass_utils import run_bass_kernel_spmd

    _ensure_trace_hook_importable()
    nc = _get_nc()
    in_maps = make_in_maps(x, idx, w_sparse, b_sparse, w_motor, b_motor)
    res = run_bass_kernel_spmd(nc, in_maps, core_ids=list(range(N_CORES)))
    _CACHE["last_results"] = res
    return combine_outputs([res.results[k]["out"] for k in range(N_CORES)])


# revision 13
# speedup vs baseline: 1.5546x; 1.0257x over previous
"""BrainModel kernel for 8 TRN2 NeuronCores (raw bass, no Tile).

Reference computation:
    gathered = x[:, idx]                              # [B, O, C]
    pre = einsum('boc,oc->bo', gathered, w_sparse) + b_sparse
    new_x = sigmoid(pre)                              # [B, O]
    q = new_x[:, -N_MOTORS:] @ w_motor.T + b_motor    # [B, A]

Only the last N_MOTORS=256 rows of idx/w_sparse/b_sparse reach q, so the
other 98720 output neurons are dead code. We shard those 256 motor
neurons across the 8 cores (32 each -> 1024 gathered x-columns per core).

The gather is 8 SWDGE indirect DMAs of 128 descriptors each (measured HW
facts: the indirect path consumes exactly ONE index per partition per
instruction - multi-column offset APs are ignored/garbage, dma_gather's
Q7 ucode is 30x slower per descriptor plus a 9us library load, and HWDGE
engines crash on indirect DMAs - so 8 x ~1.1us serialized emission on
the Pool sequencer is the floor).

v4 over the original baseline:
  - x table in bf16 [100000, 64]: gather descriptors move 128B instead
    of 256B, and all matmuls run single-pass bf16 (the fp32 path pumps
    every matmul twice as LOW/HIGH). rel err ~3e-3, tolerance is 2e-2.
  - two warmup gathers (row 0 x 128 into scratch) keep the Q7 SWDGE
    busy from preamble exit until the index tile lands, eating the
    SWDGE cold-start + post-wait dispatch stall (~1-2us combined).
  - sigmoid outputs bf16 so the motor matmul is also single-pass.
Host sums the 8 partials and transposes to [B, A].
"""

from contextlib import ExitStack

import ml_dtypes
import numpy as np

import concourse.bass as bass
from concourse import mybir

N_NEURONS = 100000
N_MOTORS = 256
N_CONN = 32
N_ACT = 16
BATCH = 64
N_CORES = 8
M_PER_CORE = N_MOTORS // N_CORES  # 32 motor neurons per core
R = M_PER_CORE * N_CONN  # 1024 gathered x-rows per core
P = 128  # SBUF partitions
J = R // P  # 8 gather/matmul chunks

C_WK = J * M_PER_CORE // 2  # 128: f32 cols of bf16 Wk
C_WMT = C_WK + N_ACT // 2  # 136: wmT end (bf16 [32, 16])
C_BS = C_WMT  # 136: b_sparse col
C_BM = C_BS + 1  # 137: b_motor/8 col
C_IDX = C_BM + 1  # 138: idx cols (8 x int32)
AUXC = C_IDX + J  # 146

BF16 = ml_dtypes.bfloat16

_CACHE: dict = {}


def _build_nc() -> bass.Bass:
    f32 = mybir.dt.float32
    bf16 = mybir.dt.bfloat16
    i32 = mybir.dt.int32
    nc = bass.Bass(enable_partition_id=False)

    tbl = nc.declare_dram_parameter("tbl", [N_NEURONS, BATCH], bf16, isOutput=False)
    aux1 = nc.declare_dram_parameter("aux1", [P, J], f32, isOutput=False)
    aux2 = nc.declare_dram_parameter("aux2", [P, C_IDX], f32, isOutput=False)
    out = nc.declare_dram_parameter("out", [N_ACT, BATCH], f32, isOutput=True)

    with ExitStack() as ctx:
        aux1_sb = ctx.enter_context(nc.sbuf_tensor("aux1_sb", [P, J], f32))
        aux_sb = ctx.enter_context(nc.sbuf_tensor("aux_sb", [P, C_IDX], f32))
        G = ctx.enter_context(nc.sbuf_tensor("G", [P, J * BATCH], bf16))
        wscr = ctx.enter_context(nc.sbuf_tensor("wscr", [P, BATCH], bf16))
        widx = ctx.enter_context(nc.sbuf_tensor("widx", [P, 1], i32))
        s_sb = ctx.enter_context(nc.sbuf_tensor("s_sb", [M_PER_CORE, BATCH], bf16))
        q_sb = ctx.enter_context(nc.sbuf_tensor("q_sb", [N_ACT, BATCH], f32))
        pre_ps = ctx.enter_context(nc.psum_tensor("pre_ps", [M_PER_CORE, BATCH], f32))
        q_ps = ctx.enter_context(nc.psum_tensor("q_ps", [N_ACT, BATCH], f32))
        isem = ctx.enter_context(nc.semaphore("isem"))
        wsem = ctx.enter_context(nc.semaphore("wsem"))
        wgsem = ctx.enter_context(nc.semaphore("wgsem"))
        odma_sem = ctx.enter_context(nc.semaphore("odma_sem"))
        # One completion sem per gather chunk: a single shared sem would be
        # racy -- each DMA's 16 increments come from 16 independent SDMA
        # engines, so a running count can reach 16*(j+1) before chunk j has
        # fully landed.
        gdma_sems = [
            ctx.enter_context(nc.semaphore(f"gdma_sem{j}")) for j in range(J)
        ]
        pe_sem = ctx.enter_context(nc.semaphore("pe_sem"))
        act_sem = ctx.enter_context(nc.semaphore("act_sem"))
        warm_sb = ctx.enter_context(nc.sbuf_tensor("warm_sb", [1, 1], f32))
        pad_sb = ctx.enter_context(nc.sbuf_tensor("pad_sb", [1, 1], f32))
        block = ctx.enter_context(nc.Block())

        @block.sync
        def _(sync):
            # idx first (small, contiguous) so the gathers start ASAP; weights
            # on their own sem (completion order of two DMAs is unordered).
            sync.dma_start(out=aux1_sb[:], in_=aux1[:]).then_inc(isem, 16)
            sync.dma_start(out=aux_sb[:], in_=aux2[:]).then_inc(wsem, 16)
            sync.wait_ge(odma_sem, 16)

        @block.gpsimd
        def _(gpsimd):
            # Warmup gather (row 0 into scratch): eats the SWDGE cold-start
            # stall inside the idx-DMA wait window, ending just before the
            # idx tile lands so the first real gather dispatches hot.
            gpsimd.memset(widx[:], 0)
            gpsimd.indirect_dma_start(
                out=wscr[:],
                out_offset=None,
                in_=tbl[:],
                in_offset=bass.IndirectOffsetOnAxis(ap=widx[:], axis=0),
            ).then_inc(wgsem, 16)
            gpsimd.wait_ge(isem, 16)
            # Cheap op right after the wait absorbs part of the Pool post-wait
            # dispatch stall.
            gpsimd.memset(pad_sb[:], 0)
            # One index per partition per instruction: partition p of the dest
            # gets dest-free-size contiguous bytes starting at tbl row idx[p].
            for j in range(J):
                gpsimd.indirect_dma_start(
                    out=G[:, j * BATCH : (j + 1) * BATCH],
                    out_offset=None,
                    in_=tbl[:],
                    in_offset=bass.IndirectOffsetOnAxis(
                        ap=aux1_sb[:, j : j + 1].bitcast(i32),
                        axis=0,
                    ),
                ).then_inc(gdma_sems[j], 16)

        @block.tensor
        def _(tensor):
            tensor.wait_ge(wsem, 16)
            wk = aux_sb[:, :C_WK].bitcast(mybir.dt.bfloat16)  # [128, J*32]
            # pre[m, b] = sum_{p,j} Wk[p, j*32+m] * x[b, idx_flat[p*J+j]]
            for j in range(J):
                tensor.wait_ge(gdma_sems[j], 16)
                mm = tensor.matmul(
                    pre_ps[:],
                    wk[:, j * M_PER_CORE : (j + 1) * M_PER_CORE],
                    G[:, j * BATCH : (j + 1) * BATCH],
                    start=(j == 0),
                    stop=(j == J - 1),
                )
            mm.then_inc(pe_sem, 1)
            tensor.wait_ge(act_sem, 1)
            # q_part[a, b] = sum_m wmT[m, a] * s[m, b]
            tensor.matmul(
                q_ps[:],
                aux_sb[:M_PER_CORE, C_WK:C_WMT].bitcast(mybir.dt.bfloat16),
                s_sb[:],
                start=True,
                stop=True,
            ).then_inc(pe_sem, 1)

        @block.scalar
        def _(scalar):
            # Dummy activation preloads the sigmoid LUT off the critical path
            # (the table load is ~1.3us and otherwise serializes after the
            # last matmul). Reads its own scratch; the value is irrelevant.
            scalar.activation(
                warm_sb[:], warm_sb[:], mybir.ActivationFunctionType.Sigmoid
            )
            scalar.wait_ge(pe_sem, 1)
            # s = sigmoid(pre + b_sparse), bf16 out for the 1-pass motor matmul
            scalar.activation(
                s_sb[:],
                pre_ps[:],
                mybir.ActivationFunctionType.Sigmoid,
                bias=aux_sb[:M_PER_CORE, C_BS : C_BS + 1],
            ).then_inc(act_sem, 1)
            scalar.wait_ge(pe_sem, 2)
            # q_sb = q_ps + b_motor/8 (PSUM -> SBUF)
            scalar.activation(
                q_sb[:],
                q_ps[:],
                mybir.ActivationFunctionType.Identity,
                bias=aux_sb[:N_ACT, C_BM : C_BM + 1],
            )
            # ScalarE is HWDGE-capable: issue the output DMA right here,
            # skipping a cross-engine semaphore hop to Sync.
            scalar.dma_start(out=out[:], in_=q_sb[:]).then_inc(odma_sem, 16)

    return nc


def _get_nc() -> bass.Bass:
    if "nc" not in _CACHE:
        _CACHE["nc"] = _build_nc()
    return _CACHE["nc"]


def make_in_maps(x, idx, w_sparse, b_sparse, w_motor, b_motor):
    """Shard FULL inputs into the 8 per-core input dicts."""
    x = np.asarray(x, dtype=np.float32)
    idx_m = np.asarray(idx)[-N_MOTORS:].astype(np.int32)  # [256, 32]
    w_m = np.asarray(w_sparse, dtype=np.float32)[-N_MOTORS:]  # [256, 32]
    b_m = np.asarray(b_sparse, dtype=np.float32)[-N_MOTORS:]  # [256]
    wm = np.asarray(w_motor, dtype=np.float32)  # [16, 256]
    bm = np.asarray(b_motor, dtype=np.float32)  # [16]

    tbl = np.ascontiguousarray(x.T.astype(BF16))  # [N_NEURONS, B], row n = x[:, n]

    # flat position r = p*J + j: gathered row lands at G[p, j*B:(j+1)*B]; its
    # weight multiplies motor column m = r//32 of matmul chunk j.
    r = np.arange(R)
    o_l, c = r // N_CONN, r % N_CONN
    p_r, j_r = r // J, r % J

    in_maps = []
    for k in range(N_CORES):
        rows = slice(k * M_PER_CORE, (k + 1) * M_PER_CORE)
        w_core = w_m[rows]  # [32, 32]

        aux1 = np.ascontiguousarray(idx_m[rows].reshape(P, J)).view(np.float32)

        aux2 = np.zeros((P, C_IDX), np.float32)
        wk = np.zeros((P, J * M_PER_CORE), BF16)
        wk[p_r, j_r * M_PER_CORE + o_l] = w_core[o_l, c].astype(BF16)
        aux2[:, :C_WK] = wk.view(np.float32)
        aux2[:M_PER_CORE, C_WK:C_WMT] = (
            np.ascontiguousarray(wm[:, rows].T.astype(BF16))
            .view(np.float32)
            .reshape(M_PER_CORE, N_ACT // 2)
        )
        aux2[:M_PER_CORE, C_BS] = b_m[rows]
        aux2[:N_ACT, C_BM] = bm / N_CORES

        in_maps.append({"tbl": tbl, "aux1": aux1, "aux2": aux2})
    return in_maps


def combine_outputs(partials):
    """Reduce the 8 per-core [A, B] partials to the full [B, A] output."""
    q = np.sum(np.stack(partials, axis=0), axis=0, dtype=np.float64)
    return np.ascontiguousarray(q.T).astype(np.float32)


def _ensure_trace_hook_importable():
    """bass_utils' axon trace path imports antenv.axon_hooks; some containers
    ship an antenv without it. Provide a null hook so trace degrades to a
    plain run instead of crashing."""
    import os

    if not os.environ.get("BASS_TRACE"):
        return
    try:
        import antenv.axon_hooks  # noqa: F401
    except ImportError:
        import sys
        import types

        import antenv

        m = types.ModuleType("antenv.axon_hooks")
        state = {"hook": None}
        m.set_axon_ntff_profile_hook = lambda h: state.__setitem__("hook", h)
        m.get_axon_ntff_profile_hook = lambda: state["hook"]
        sys.modules["antenv.axon_hooks"] = m
        antenv.axon_hooks = m


def kernel(x, idx, w_sparse, b_sparse, w_motor, b_motor):
    from concourse.bass_utils import run_bass_kernel_spmd

    _ensure_trace_hook_importable()
    nc = _get_nc()
    in_maps = make_in_maps(x, idx, w_sparse, b_sparse, w_motor, b_motor)
    res = run_bass_kernel_spmd(nc, in_maps, core_ids=list(range(N_CORES)))
    _CACHE["last_results"] = res
    return combine_outputs([res.results[k]["out"] for k in range(N_CORES)])


# revision 15
# speedup vs baseline: 1.6075x; 1.0341x over previous
"""BrainModel kernel for 8 TRN2 NeuronCores (raw bass, no Tile).

Reference computation:
    gathered = x[:, idx]                              # [B, O, C]
    pre = einsum('boc,oc->bo', gathered, w_sparse) + b_sparse
    new_x = sigmoid(pre)                              # [B, O]
    q = new_x[:, -N_MOTORS:] @ w_motor.T + b_motor    # [B, A]

Only the last N_MOTORS=256 rows of idx/w_sparse/b_sparse reach q, so the
other 98720 output neurons are dead code. We shard those 256 motor
neurons across the 8 cores (32 each -> 1024 gathered x-columns per core).

The gather is 8 SWDGE indirect DMAs of 128 descriptors each (measured HW
facts: the indirect path consumes exactly ONE index per partition per
instruction - multi-column offset APs are ignored/garbage, dma_gather's
Q7 ucode is 30x slower per descriptor plus a 9us library load, and HWDGE
engines crash on indirect DMAs - so 8 x ~1.1us serialized emission on
the Pool sequencer is the floor).

v4 over the original baseline:
  - x table in bf16 [100000, 64]: gather descriptors move 128B instead
    of 256B, and all matmuls run single-pass bf16 (the fp32 path pumps
    every matmul twice as LOW/HIGH). rel err ~3e-3, tolerance is 2e-2.
  - two warmup gathers (row 0 x 128 into scratch) keep the Q7 SWDGE
    busy from preamble exit until the index tile lands, eating the
    SWDGE cold-start + post-wait dispatch stall (~1-2us combined).
  - sigmoid outputs bf16 so the motor matmul is also single-pass.
Host sums the 8 partials and transposes to [B, A].
"""

from contextlib import ExitStack

import ml_dtypes
import numpy as np

import concourse.bass as bass
from concourse import mybir

N_NEURONS = 100000
N_MOTORS = 256
N_CONN = 32
N_ACT = 16
BATCH = 64
N_CORES = 8
M_PER_CORE = N_MOTORS // N_CORES  # 32 motor neurons per core
R = M_PER_CORE * N_CONN  # 1024 gathered x-rows per core
P = 128  # SBUF partitions
J = R // P  # 8 gather/matmul chunks

C_WK = J * M_PER_CORE // 2  # 128: f32 cols of bf16 Wk
C_WMT = C_WK + N_ACT // 2  # 136: wmT end (bf16 [32, 16])
C_BS = C_WMT  # 136: b_sparse col
C_BM = C_BS + 1  # 137: b_motor/8 col
C_IDX = C_BM + 1  # 138: idx cols (8 x int32)
AUXC = C_IDX + J  # 146

BF16 = ml_dtypes.bfloat16

_CACHE: dict = {}


def _build_nc() -> bass.Bass:
    f32 = mybir.dt.float32
    bf16 = mybir.dt.bfloat16
    i32 = mybir.dt.int32
    nc = bass.Bass(enable_partition_id=False)

    # Drop the init-emitted const-AP memsets from the Pool stream: they sit on
    # the preamble's critical path (~0.4us before the barrier releases) and
    # nothing in this kernel reads the const tiles (all-bf16 matmuls, explicit
    # AP biases, no bounds checks).
    blk = nc.m.functions[0].blocks[0]
    blk.instructions = [i for i in blk.instructions if i.opcode != "Memset"]

    tbl = nc.declare_dram_parameter("tbl", [N_NEURONS, BATCH], bf16, isOutput=False)
    aux1 = nc.declare_dram_parameter("aux1", [P, J], f32, isOutput=False)
    aux2 = nc.declare_dram_parameter("aux2", [P, C_IDX], f32, isOutput=False)
    out = nc.declare_dram_parameter("out", [N_ACT, BATCH], f32, isOutput=True)

    with ExitStack() as ctx:
        aux1_sb = ctx.enter_context(nc.sbuf_tensor("aux1_sb", [P, J], f32))
        aux_sb = ctx.enter_context(nc.sbuf_tensor("aux_sb", [P, C_IDX], f32))
        G = ctx.enter_context(nc.sbuf_tensor("G", [P, J * BATCH], bf16))
        wscr = ctx.enter_context(nc.sbuf_tensor("wscr", [P, BATCH], bf16))
        widx = ctx.enter_context(nc.sbuf_tensor("widx", [P, 1], i32))
        s_sb = ctx.enter_context(nc.sbuf_tensor("s_sb", [M_PER_CORE, BATCH], bf16))
        q_sb = ctx.enter_context(nc.sbuf_tensor("q_sb", [N_ACT, BATCH], f32))
        pre_ps = ctx.enter_context(nc.psum_tensor("pre_ps", [M_PER_CORE, BATCH], f32))
        q_ps = ctx.enter_context(nc.psum_tensor("q_ps", [N_ACT, BATCH], f32))
        isem = ctx.enter_context(nc.semaphore("isem"))
        wsem = ctx.enter_context(nc.semaphore("wsem"))
        wgsem = ctx.enter_context(nc.semaphore("wgsem"))
        odma_sem = ctx.enter_context(nc.semaphore("odma_sem"))
        # One completion sem per gather chunk: a single shared sem would be
        # racy -- each DMA's 16 increments come from 16 independent SDMA
        # engines, so a running count can reach 16*(j+1) before chunk j has
        # fully landed.
        gdma_sems = [
            ctx.enter_context(nc.semaphore(f"gdma_sem{j}")) for j in range(J)
        ]
        pe_sem = ctx.enter_context(nc.semaphore("pe_sem"))
        act_sem = ctx.enter_context(nc.semaphore("act_sem"))
        warm_sb = ctx.enter_context(nc.sbuf_tensor("warm_sb", [1, 1], f32))
        pad_sb = ctx.enter_context(nc.sbuf_tensor("pad_sb", [1, 1], f32))
        block = ctx.enter_context(nc.Block())

        @block.sync
        def _(sync):
            # idx first (small, contiguous) so the gathers start ASAP; weights
            # on their own sem (completion order of two DMAs is unordered).
            sync.dma_start(out=aux1_sb[:], in_=aux1[:]).then_inc(isem, 16)
            sync.dma_start(out=aux_sb[:], in_=aux2[:]).then_inc(wsem, 16)
            sync.wait_ge(odma_sem, 16)

        @block.gpsimd
        def _(gpsimd):
            # Warmup gather (row 0 into scratch): eats the SWDGE cold-start
            # stall inside the idx-DMA wait window, ending just before the
            # idx tile lands so the first real gather dispatches hot.
            gpsimd.memset(widx[:], 0)
            gpsimd.indirect_dma_start(
                out=wscr[:],
                out_offset=None,
                in_=tbl[:],
                in_offset=bass.IndirectOffsetOnAxis(ap=widx[:], axis=0),
            ).then_inc(wgsem, 16)
            gpsimd.wait_ge(isem, 16)
            # Cheap op right after the wait absorbs part of the Pool post-wait
            # dispatch stall.
            gpsimd.memset(pad_sb[:], 0)
            # One index per partition per instruction: partition p of the dest
            # gets dest-free-size contiguous bytes starting at tbl row idx[p].
            for j in range(J):
                gpsimd.indirect_dma_start(
                    out=G[:, j * BATCH : (j + 1) * BATCH],
                    out_offset=None,
                    in_=tbl[:],
                    in_offset=bass.IndirectOffsetOnAxis(
                        ap=aux1_sb[:, j : j + 1].bitcast(i32),
                        axis=0,
                    ),
                ).then_inc(gdma_sems[j], 16)

        @block.tensor
        def _(tensor):
            tensor.wait_ge(wsem, 16)
            wk = aux_sb[:, :C_WK].bitcast(mybir.dt.bfloat16)  # [128, J*32]
            # pre[m, b] = sum_{p,j} Wk[p, j*32+m] * x[b, idx_flat[p*J+j]]
            for j in range(J):
                tensor.wait_ge(gdma_sems[j], 16)
                mm = tensor.matmul(
                    pre_ps[:],
                    wk[:, j * M_PER_CORE : (j + 1) * M_PER_CORE],
                    G[:, j * BATCH : (j + 1) * BATCH],
                    start=(j == 0),
                    stop=(j == J - 1),
                )
            mm.then_inc(pe_sem, 1)
            tensor.wait_ge(act_sem, 1)
            # q_part[a, b] = sum_m wmT[m, a] * s[m, b]
            tensor.matmul(
                q_ps[:],
                aux_sb[:M_PER_CORE, C_WK:C_WMT].bitcast(mybir.dt.bfloat16),
                s_sb[:],
                start=True,
                stop=True,
            ).then_inc(pe_sem, 1)

        @block.scalar
        def _(scalar):
            # Dummy activation preloads the sigmoid LUT off the critical path
            # (the table load is ~1.3us and otherwise serializes after the
            # last matmul). Reads its own scratch; the value is irrelevant.
            scalar.activation(
                warm_sb[:],
                warm_sb[:],
                mybir.ActivationFunctionType.Sigmoid,
                bias=warm_sb[:],  # explicit AP: avoids the const-0 tile
            )
            scalar.wait_ge(pe_sem, 1)
            # s = sigmoid(pre + b_sparse), bf16 out for the 1-pass motor matmul
            scalar.activation(
                s_sb[:],
                pre_ps[:],
                mybir.ActivationFunctionType.Sigmoid,
                bias=aux_sb[:M_PER_CORE, C_BS : C_BS + 1],
            ).then_inc(act_sem, 1)
            scalar.wait_ge(pe_sem, 2)
            # q_sb = q_ps + b_motor/8 (PSUM -> SBUF)
            scalar.activation(
                q_sb[:],
                q_ps[:],
                mybir.ActivationFunctionType.Identity,
                bias=aux_sb[:N_ACT, C_BM : C_BM + 1],
            )
            # ScalarE is HWDGE-capable: issue the output DMA right here,
            # skipping a cross-engine semaphore hop to Sync.
            scalar.dma_start(out=out[:], in_=q_sb[:]).then_inc(odma_sem, 16)

    return nc


def _get_nc() -> bass.Bass:
    if "nc" not in _CACHE:
        _CACHE["nc"] = _build_nc()
    return _CACHE["nc"]


def make_in_maps(x, idx, w_sparse, b_sparse, w_motor, b_motor):
    """Shard FULL inputs into the 8 per-core input dicts."""
    x = np.asarray(x, dtype=np.float32)
    idx_m = np.asarray(idx)[-N_MOTORS:].astype(np.int32)  # [256, 32]
    w_m = np.asarray(w_sparse, dtype=np.float32)[-N_MOTORS:]  # [256, 32]
    b_m = np.asarray(b_sparse, dtype=np.float32)[-N_MOTORS:]  # [256]
    wm = np.asarray(w_motor, dtype=np.float32)  # [16, 256]
    bm = np.asarray(b_motor, dtype=np.float32)  # [16]

    tbl = np.ascontiguousarray(x.T.astype(BF16))  # [N_NEURONS, B], row n = x[:, n]

    # flat position r = p*J + j: gathered row lands at G[p, j*B:(j+1)*B]; its
    # weight multiplies motor column m = r//32 of matmul chunk j.
    r = np.arange(R)
    o_l, c = r // N_CONN, r % N_CONN
    p_r, j_r = r // J, r % J

    in_maps = []
    for k in range(N_CORES):
        rows = slice(k * M_PER_CORE, (k + 1) * M_PER_CORE)
        w_core = w_m[rows]  # [32, 32]

        aux1 = np.ascontiguousarray(idx_m[rows].reshape(P, J)).view(np.float32)

        aux2 = np.zeros((P, C_IDX), np.float32)
        wk = np.zeros((P, J * M_PER_CORE), BF16)
        wk[p_r, j_r * M_PER_CORE + o_l] = w_core[o_l, c].astype(BF16)
        aux2[:, :C_WK] = wk.view(np.float32)
        aux2[:M_PER_CORE, C_WK:C_WMT] = (
            np.ascontiguousarray(wm[:, rows].T.astype(BF16))
            .view(np.float32)
            .reshape(M_PER_CORE, N_ACT // 2)
        )
        aux2[:M_PER_CORE, C_BS] = b_m[rows]
        aux2[:N_ACT, C_BM] = bm / N_CORES

        in_maps.append({"tbl": tbl, "aux1": aux1, "aux2": aux2})
    return in_maps


def combine_outputs(partials):
    """Reduce the 8 per-core [A, B] partials to the full [B, A] output."""
    q = np.sum(np.stack(partials, axis=0), axis=0, dtype=np.float64)
    return np.ascontiguousarray(q.T).astype(np.float32)


def _ensure_trace_hook_importable():
    """bass_utils' axon trace path imports antenv.axon_hooks; some containers
    ship an antenv without it. Provide a null hook so trace degrades to a
    plain run instead of crashing."""
    import os

    if not os.environ.get("BASS_TRACE"):
        return
    try:
        import antenv.axon_hooks  # noqa: F401
    except ImportError:
        import sys
        import types

        import antenv

        m = types.ModuleType("antenv.axon_hooks")
        state = {"hook": None}
        m.set_axon_ntff_profile_hook = lambda h: state.__setitem__("hook", h)
        m.get_axon_ntff_profile_hook = lambda: state["hook"]
        sys.modules["antenv.axon_hooks"] = m
        antenv.axon_hooks = m


def kernel(x, idx, w_sparse, b_sparse, w_motor, b_motor):
    from concourse.bass_utils import run_bass_kernel_spmd

    _ensure_trace_hook_importable()
    nc = _get_nc()
    in_maps = make_in_maps(x, idx, w_sparse, b_sparse, w_motor, b_motor)
    res = run_bass_kernel_spmd(nc, in_maps, core_ids=list(range(N_CORES)))
    _CACHE["last_results"] = res
    return combine_outputs([res.results[k]["out"] for k in range(N_CORES)])


# revision 17
# speedup vs baseline: 1.6223x; 1.0092x over previous
"""BrainModel kernel for 8 TRN2 NeuronCores (raw bass, no Tile).

Reference computation:
    gathered = x[:, idx]                              # [B, O, C]
    pre = einsum('boc,oc->bo', gathered, w_sparse) + b_sparse
    new_x = sigmoid(pre)                              # [B, O]
    q = new_x[:, -N_MOTORS:] @ w_motor.T + b_motor    # [B, A]

Only the last N_MOTORS=256 rows of idx/w_sparse/b_sparse reach q, so the
other 98720 output neurons are dead code. We shard those 256 motor
neurons across the 8 cores (32 each -> 1024 gathered x-columns per core).

The gather is 8 SWDGE indirect DMAs of 128 descriptors each (measured HW
facts: the indirect path consumes exactly ONE index per partition per
instruction - multi-column offset APs are ignored/garbage, dma_gather's
Q7 ucode is 30x slower per descriptor plus a 9us library load, and HWDGE
engines crash on indirect DMAs - so 8 x ~1.1us serialized emission on
the Pool sequencer is the floor).

v4 over the original baseline:
  - x table in bf16 [100000, 64]: gather descriptors move 128B instead
    of 256B, and all matmuls run single-pass bf16 (the fp32 path pumps
    every matmul twice as LOW/HIGH). rel err ~3e-3, tolerance is 2e-2.
  - two warmup gathers (row 0 x 128 into scratch) keep the Q7 SWDGE
    busy from preamble exit until the index tile lands, eating the
    SWDGE cold-start + post-wait dispatch stall (~1-2us combined).
  - sigmoid outputs bf16 so the motor matmul is also single-pass.
Host sums the 8 partials and transposes to [B, A].
"""

from contextlib import ExitStack

import ml_dtypes
import numpy as np

import concourse.bass as bass
from concourse import mybir

N_NEURONS = 100000
N_MOTORS = 256
N_CONN = 32
N_ACT = 16
BATCH = 64
N_CORES = 8
M_PER_CORE = N_MOTORS // N_CORES  # 32 motor neurons per core
R = M_PER_CORE * N_CONN  # 1024 gathered x-rows per core
P = 128  # SBUF partitions
J = R // P  # 8 gather/matmul chunks

C_WK = J * M_PER_CORE // 2  # 128: f32 cols of bf16 Wk
C_WMT = C_WK + N_ACT // 2  # 136: wmT end (bf16 [32, 16])
C_BS = C_WMT  # 136: b_sparse col
C_BM = C_BS + 1  # 137: b_motor/8 col
C_IDX = C_BM + 1  # 138: idx cols (8 x int32)
AUXC = C_IDX + J  # 146

BF16 = ml_dtypes.bfloat16

_CACHE: dict = {}


def _build_nc() -> bass.Bass:
    f32 = mybir.dt.float32
    bf16 = mybir.dt.bfloat16
    i32 = mybir.dt.int32
    nc = bass.Bass(enable_partition_id=False)

    # Drop the init-emitted const-AP memsets from the Pool stream: they sit on
    # the preamble's critical path (~0.4us before the barrier releases) and
    # nothing in this kernel reads the const tiles (all-bf16 matmuls, explicit
    # AP biases, no bounds checks).
    blk = nc.m.functions[0].blocks[0]
    blk.instructions = [i for i in blk.instructions if i.opcode != "Memset"]

    tbl = nc.declare_dram_parameter("tbl", [N_NEURONS, BATCH], bf16, isOutput=False)
    aux1 = nc.declare_dram_parameter("aux1", [P, J], f32, isOutput=False)
    aux2 = nc.declare_dram_parameter("aux2", [P, C_IDX], f32, isOutput=False)
    out = nc.declare_dram_parameter("out", [N_ACT, BATCH], f32, isOutput=True)

    with ExitStack() as ctx:
        aux1_sb = ctx.enter_context(nc.sbuf_tensor("aux1_sb", [P, J], f32))
        aux_sb = ctx.enter_context(nc.sbuf_tensor("aux_sb", [P, C_IDX], f32))
        G = ctx.enter_context(nc.sbuf_tensor("G", [P, J * BATCH], bf16))
        wscr = ctx.enter_context(nc.sbuf_tensor("wscr", [P, BATCH], bf16))
        widx = ctx.enter_context(nc.sbuf_tensor("widx", [P, 1], i32))
        s_sb = ctx.enter_context(nc.sbuf_tensor("s_sb", [M_PER_CORE, BATCH], bf16))
        q_sb = ctx.enter_context(nc.sbuf_tensor("q_sb", [N_ACT, BATCH], f32))
        pre_ps = ctx.enter_context(nc.psum_tensor("pre_ps", [M_PER_CORE, BATCH], f32))
        q_ps = ctx.enter_context(nc.psum_tensor("q_ps", [N_ACT, BATCH], f32))
        isem = ctx.enter_context(nc.semaphore("isem"))
        wsem = ctx.enter_context(nc.semaphore("wsem"))
        wgsem = ctx.enter_context(nc.semaphore("wgsem"))
        odma_sem = ctx.enter_context(nc.semaphore("odma_sem"))
        # One completion sem per gather chunk: a single shared sem would be
        # racy -- each DMA's 16 increments come from 16 independent SDMA
        # engines, so a running count can reach 16*(j+1) before chunk j has
        # fully landed.
        gdma_sems = [
            ctx.enter_context(nc.semaphore(f"gdma_sem{j}")) for j in range(J)
        ]
        pe_sem = ctx.enter_context(nc.semaphore("pe_sem"))
        act_sem = ctx.enter_context(nc.semaphore("act_sem"))
        warm_sb = ctx.enter_context(nc.sbuf_tensor("warm_sb", [1, 1], f32))
        block = ctx.enter_context(nc.Block())

        @block.sync
        def _(sync):
            # idx first (small, contiguous) so the gathers start ASAP; weights
            # on their own sem (completion order of two DMAs is unordered).
            sync.dma_start(out=aux1_sb[:], in_=aux1[:]).then_inc(isem, 16)
            sync.dma_start(out=aux_sb[:], in_=aux2[:]).then_inc(wsem, 16)
            sync.wait_ge(odma_sem, 16)

        @block.gpsimd
        def _(gpsimd):
            # Warmup gather (row 0 into scratch): eats the SWDGE cold-start
            # stall inside the idx-DMA wait window, ending just before the
            # idx tile lands so the first real gather dispatches hot.
            gpsimd.memset(widx[:], 0)
            gpsimd.indirect_dma_start(
                out=wscr[:],
                out_offset=None,
                in_=tbl[:],
                in_offset=bass.IndirectOffsetOnAxis(ap=widx[:], axis=0),
            ).then_inc(wgsem, 16)
            gpsimd.wait_ge(isem, 16)
            # One index per partition per instruction: partition p of the dest
            # gets dest-free-size contiguous bytes starting at tbl row idx[p].
            for j in range(J):
                gpsimd.indirect_dma_start(
                    out=G[:, j * BATCH : (j + 1) * BATCH],
                    out_offset=None,
                    in_=tbl[:],
                    in_offset=bass.IndirectOffsetOnAxis(
                        ap=aux1_sb[:, j : j + 1].bitcast(i32),
                        axis=0,
                    ),
                ).then_inc(gdma_sems[j], 16)

        @block.tensor
        def _(tensor):
            tensor.wait_ge(wsem, 16)
            wk = aux_sb[:, :C_WK].bitcast(mybir.dt.bfloat16)  # [128, J*32]
            # pre[m, b] = sum_{p,j} Wk[p, j*32+m] * x[b, idx_flat[p*J+j]]
            for j in range(J):
                tensor.wait_ge(gdma_sems[j], 16)
                mm = tensor.matmul(
                    pre_ps[:],
                    wk[:, j * M_PER_CORE : (j + 1) * M_PER_CORE],
                    G[:, j * BATCH : (j + 1) * BATCH],
                    start=(j == 0),
                    stop=(j == J - 1),
                )
            mm.then_inc(pe_sem, 1)
            tensor.wait_ge(act_sem, 1)
            # q_part[a, b] = sum_m wmT[m, a] * s[m, b]
            tensor.matmul(
                q_ps[:],
                aux_sb[:M_PER_CORE, C_WK:C_WMT].bitcast(mybir.dt.bfloat16),
                s_sb[:],
                start=True,
                stop=True,
            ).then_inc(pe_sem, 1)

        @block.scalar
        def _(scalar):
            # Dummy activation preloads the sigmoid LUT off the critical path
            # (the table load is ~1.3us and otherwise serializes after the
            # last matmul). Reads its own scratch; the value is irrelevant.
            scalar.activation(
                warm_sb[:],
                warm_sb[:],
                mybir.ActivationFunctionType.Sigmoid,
                bias=warm_sb[:],  # explicit AP: avoids the const-0 tile
            )
            scalar.wait_ge(pe_sem, 1)
            # s = sigmoid(pre + b_sparse), bf16 out for the 1-pass motor matmul
            scalar.activation(
                s_sb[:],
                pre_ps[:],
                mybir.ActivationFunctionType.Sigmoid,
                bias=aux_sb[:M_PER_CORE, C_BS : C_BS + 1],
            ).then_inc(act_sem, 1)
            scalar.wait_ge(pe_sem, 2)
            # q_sb = q_ps + b_motor/8 (PSUM -> SBUF)
            scalar.activation(
                q_sb[:],
                q_ps[:],
                mybir.ActivationFunctionType.Identity,
                bias=aux_sb[:N_ACT, C_BM : C_BM + 1],
            )
            # ScalarE is HWDGE-capable: issue the output DMA right here,
            # skipping a cross-engine semaphore hop to Sync.
            scalar.dma_start(out=out[:], in_=q_sb[:]).then_inc(odma_sem, 16)

    return nc


def _get_nc() -> bass.Bass:
    if "nc" not in _CACHE:
        _CACHE["nc"] = _build_nc()
    return _CACHE["nc"]


def make_in_maps(x, idx, w_sparse, b_sparse, w_motor, b_motor):
    """Shard FULL inputs into the 8 per-core input dicts."""
    x = np.asarray(x, dtype=np.float32)
    idx_m = np.asarray(idx)[-N_MOTORS:].astype(np.int32)  # [256, 32]
    w_m = np.asarray(w_sparse, dtype=np.float32)[-N_MOTORS:]  # [256, 32]
    b_m = np.asarray(b_sparse, dtype=np.float32)[-N_MOTORS:]  # [256]
    wm = np.asarray(w_motor, dtype=np.float32)  # [16, 256]
    bm = np.asarray(b_motor, dtype=np.float32)  # [16]

    tbl = np.ascontiguousarray(x.T.astype(BF16))  # [N_NEURONS, B], row n = x[:, n]

    # flat position r = p*J + j: gathered row lands at G[p, j*B:(j+1)*B]; its
    # weight multiplies motor column m = r//32 of matmul chunk j.
    r = np.arange(R)
    o_l, c = r // N_CONN, r % N_CONN
    p_r, j_r = r // J, r % J

    in_maps = []
    for k in range(N_CORES):
        rows = slice(k * M_PER_CORE, (k + 1) * M_PER_CORE)
        w_core = w_m[rows]  # [32, 32]

        aux1 = np.ascontiguousarray(idx_m[rows].reshape(P, J)).view(np.float32)

        aux2 = np.zeros((P, C_IDX), np.float32)
        wk = np.zeros((P, J * M_PER_CORE), BF16)
        wk[p_r, j_r * M_PER_CORE + o_l] = w_core[o_l, c].astype(BF16)
        aux2[:, :C_WK] = wk.view(np.float32)
        aux2[:M_PER_CORE, C_WK:C_WMT] = (
            np.ascontiguousarray(wm[:, rows].T.astype(BF16))
            .view(np.float32)
            .reshape(M_PER_CORE, N_ACT // 2)
        )
        aux2[:M_PER_CORE, C_BS] = b_m[rows]
        aux2[:N_ACT, C_BM] = bm / N_CORES

        in_maps.append({"tbl": tbl, "aux1": aux1, "aux2": aux2})
    return in_maps


def combine_outputs(partials):
    """Reduce the 8 per-core [A, B] partials to the full [B, A] output."""
    q = np.sum(np.stack(partials, axis=0), axis=0, dtype=np.float64)
    return np.ascontiguousarray(q.T).astype(np.float32)


def _ensure_trace_hook_importable():
    """bass_utils' axon trace path imports antenv.axon_hooks; some containers
    ship an antenv without it. Provide a null hook so trace degrades to a
    plain run instead of crashing."""
    import os

    if not os.environ.get("BASS_TRACE"):
        return
    try:
        import antenv.axon_hooks  # noqa: F401
    except ImportError:
        import sys
        import types

        import antenv

        m = types.ModuleType("antenv.axon_hooks")
        state = {"hook": None}
        m.set_axon_ntff_profile_hook = lambda h: state.__setitem__("hook", h)
        m.get_axon_ntff_profile_hook = lambda: state["hook"]
        sys.modules["antenv.axon_hooks"] = m
        antenv.axon_hooks = m


def kernel(x, idx, w_sparse, b_sparse, w_motor, b_motor):
    from concourse.bass_utils import run_bass_kernel_spmd

    _ensure_trace_hook_importable()
    nc = _get_nc()
    in_maps = make_in_maps(x, idx, w_sparse, b_sparse, w_motor, b_motor)
    res = run_bass_kernel_spmd(nc, in_maps, core_ids=list(range(N_CORES)))
    _CACHE["last_results"] = res
    return combine_outputs([res.results[k]["out"] for k in range(N_CORES)])
